# revision 1
# baseline (speedup 1.0000x reference)
"""Multi-head attention (B=4, S=2048, H=16, D=64, C=1024) on 8 NeuronCores.

Sharding: core c handles batch b=c//2 and head-half half=c%2 (8 heads = 512
inner dims).  Each core computes q/k/v projections for its half of the heads,
full softmax attention over S=2048, and a partial output projection through
its 512 rows of Wo.  Host sums the two partials per batch and adds the bias.

Per-core kernel layout (all matmul operands bf16, PSUM accumulation fp32):
  xt    [C=1024, S=2048]   hidden_states[b].T          (host pre-transposed)
  wq/wk/wv [C, I=512]      per-half weight columns
  wo    [I=512, C=1024]    per-half weight rows
  qT,kT [I, S] stored as 4 SBUF tiles [128, 2048]  (head pair per tile)
  v_pad [S, 8*65]          v with a ones column per head (row-sum via matmul)
  scores^T per (pair, qi-chunk, kj-tile): [kj=128, qi=512] via row-tiled
  (K=64) matmul pairs; exp on ScalarE; p@[v|1] accumulated in PSUM over kj.
"""

import functools

import numpy as np
import ml_dtypes

S = 2048          # sequence length
C = 1024          # query dim
I = 512           # inner dims per core (8 heads x 64)
HC = 8            # heads per core
D = 64            # head dim
NCORES = 8
SCALE = D ** -0.5
CT = C // 128     # 8 c-tiles
IT = I // 128     # 4 i-tiles (head pairs)
ST = S // 128     # 16 s-tiles
NQ = S // 512     # 4 qi chunks
VW = D + 1        # 65: v plus ones column


def _build():
    import concourse.bacc as bacc
    import concourse.tile as tile
    from concourse import mybir

    f32 = mybir.dt.float32
    bf16 = mybir.dt.bfloat16
    Exp = mybir.ActivationFunctionType.Exp

    nc = bacc.Bacc("TRN2", target_bir_lowering=False, debug=False,
                   num_devices=NCORES)

    xt_d = nc.dram_tensor("xt", [C, S], bf16, kind="ExternalInput").ap()
    wq_d = nc.dram_tensor("wq", [C, I], bf16, kind="ExternalInput").ap()
    wk_d = nc.dram_tensor("wk", [C, I], bf16, kind="ExternalInput").ap()
    wv_d = nc.dram_tensor("wv", [C, I], bf16, kind="ExternalInput").ap()
    wo_d = nc.dram_tensor("wo", [I, C], bf16, kind="ExternalInput").ap()
    out_d = nc.dram_tensor("out", [S, C], f32, kind="ExternalOutput").ap()

    with tile.TileContext(nc) as tc:
        with (
            tc.tile_pool(name="const", bufs=1) as const,
            tc.tile_pool(name="work", bufs=3) as work,
            tc.tile_pool(name="outp", bufs=3) as outp,
            tc.tile_pool(name="ps", bufs=4, space="PSUM") as ps_pool,
            tc.tile_pool(name="pv", bufs=4, space="PSUM") as pv_pool,
        ):
            # ---- load inputs -------------------------------------------------
            xt_sb = const.tile([128, CT, S], bf16)
            xt_r = xt_d.rearrange("(t p) s -> p t s", p=128)
            for ct in range(CT):
                nc.sync.dma_start(out=xt_sb[:, ct, :], in_=xt_r[:, ct, :])

            wq_sb = const.tile([128, CT, I], bf16)
            nc.sync.dma_start(out=wq_sb, in_=wq_d.rearrange("(t p) i -> p t i", p=128))
            wk_sb = const.tile([128, CT, I], bf16)
            nc.sync.dma_start(out=wk_sb, in_=wk_d.rearrange("(t p) i -> p t i", p=128))
            wv_sb = const.tile([128, CT, I], bf16)
            nc.sync.dma_start(out=wv_sb, in_=wv_d.rearrange("(t p) i -> p t i", p=128))
            wo_sb = const.tile([128, IT, C], bf16)
            nc.sync.dma_start(out=wo_sb, in_=wo_d.rearrange("(t p) c -> p t c", p=128))

            ones_sb = const.tile([1, 128], bf16)
            nc.vector.memset(ones_sb, 1.0)

            # ---- projections -------------------------------------------------
            qT_sb = const.tile([128, IT, S], bf16)
            kT_sb = const.tile([128, IT, S], bf16)
            v_sb = const.tile([128, ST, HC * VW], bf16)
            # ones column per head for the softmax denominator
            v_ones = v_sb.rearrange("p t (h e) -> p t h e", e=VW)[:, :, :, D:D + 1]
            nc.vector.memset(v_ones, 1.0)

            for (w_sb, o_sb) in ((wq_sb, qT_sb), (wk_sb, kT_sb)):
                for it in range(IT):
                    for nq in range(NQ):
                        acc = ps_pool.tile([128, 512], f32, tag="ps", name="proj_ps")
                        for ct in range(CT):
                            nc.tensor.matmul(
                                acc,
                                lhsT=w_sb[:, ct, it * 128:(it + 1) * 128],
                                rhs=xt_sb[:, ct, nq * 512:(nq + 1) * 512],
                                start=(ct == 0), stop=(ct == CT - 1))
                        nc.vector.tensor_copy(
                            out=o_sb[:, it, nq * 512:(nq + 1) * 512], in_=acc)

            for st in range(ST):
                acc = ps_pool.tile([128, 512], f32, tag="ps", name="v_ps")
                for ct in range(CT):
                    nc.tensor.matmul(
                        acc,
                        lhsT=xt_sb[:, ct, st * 128:(st + 1) * 128],
                        rhs=wv_sb[:, ct, :],
                        start=(ct == 0), stop=(ct == CT - 1))
                for h in range(HC):
                    nc.vector.tensor_copy(
                        out=v_sb[:, st, h * VW:h * VW + D],
                        in_=acc[:, h * D:(h + 1) * D])

            # ---- attention ---------------------------------------------------
            oT_sb = const.tile([128, IT, S], bf16)
            for hp in range(IT):
                hA, hB = 2 * hp, 2 * hp + 1
                for nq in range(NQ):
                    qs = slice(nq * 512, (nq + 1) * 512)
                    oA = pv_pool.tile([VW, 512], f32, tag="pv", name="oA")
                    oB = pv_pool.tile([VW, 512], f32, tag="pv", name="oB")
                    for kt in range(ST):
                        ks = slice(kt * 128, (kt + 1) * 128)
                        sA = ps_pool.tile([128, 512], f32, tag="ps", name="sA")
                        sB = ps_pool.tile([128, 512], f32, tag="ps", name="sB")
                        nc.tensor.matmul(
                            sA, lhsT=kT_sb[0:64, hp, ks], rhs=qT_sb[0:64, hp, qs],
                            start=True, stop=True, tile_position=(0, 0))
                        nc.tensor.matmul(
                            sB, lhsT=kT_sb[64:128, hp, ks], rhs=qT_sb[64:128, hp, qs],
                            start=True, stop=True, tile_position=(64, 0))
                        pA = work.tile([128, 512], bf16, tag="p", name="pA")
                        pB = work.tile([128, 512], bf16, tag="p", name="pB")
                        nc.scalar.activation(out=pA, in_=sA, func=Exp, scale=SCALE)
                        nc.scalar.activation(out=pB, in_=sB, func=Exp, scale=SCALE)
                        nc.tensor.matmul(
                            oA, lhsT=v_sb[:, kt, hA * VW:(hA + 1) * VW], rhs=pA,
                            start=(kt == 0), stop=(kt == ST - 1))
                        nc.tensor.matmul(
                            oB, lhsT=v_sb[:, kt, hB * VW:(hB + 1) * VW], rhs=pB,
                            start=(kt == 0), stop=(kt == ST - 1))
                    # normalize: recip of the denominator row, broadcast across
                    # the 64 head dims via a K=1 matmul, multiply on VectorE
                    rA = work.tile([1, 512], mybir.dt.float32, tag="recip", name="rA")
                    rB = work.tile([1, 512], mybir.dt.float32, tag="recip", name="rB")
                    nc.vector.reciprocal(out=rA, in_=oA[D:VW, :])
                    nc.vector.reciprocal(out=rB, in_=oB[D:VW, :])
                    rAb = work.tile([1, 512], bf16, tag="recipb", name="rAb")
                    rBb = work.tile([1, 512], bf16, tag="recipb", name="rBb")
                    nc.vector.tensor_copy(out=rAb, in_=rA)
                    nc.vector.tensor_copy(out=rBb, in_=rB)
                    bc = ps_pool.tile([128, 512], f32, tag="ps", name="bc")
                    nc.tensor.matmul(bc[0:64, :], lhsT=ones_sb[0:1, 0:64],
                                     rhs=rAb, start=True, stop=True,
                                     tile_position=(0, 0))
                    nc.tensor.matmul(bc[64:128, :], lhsT=ones_sb[0:1, 0:64],
                                     rhs=rBb, start=True, stop=True,
                                     tile_position=(0, 64))
                    tA = work.tile([64, 512], mybir.dt.float32, tag="t", name="tA")
                    tB = work.tile([64, 512], mybir.dt.float32, tag="t", name="tB")
                    nc.vector.tensor_copy(out=tA, in_=oA[0:D, :])
                    nc.vector.tensor_copy(out=tB, in_=oB[0:D, :])
                    nc.vector.tensor_mul(
                        out=oT_sb[0:64, hp, qs], in0=tA, in1=bc[0:64, :])
                    nc.vector.tensor_mul(
                        out=oT_sb[64:128, hp, qs], in0=tB, in1=bc[64:128, :])

            # ---- output projection ------------------------------------------
            for st in range(ST):
                for cc in range(2):
                    acc = ps_pool.tile([128, 512], f32, tag="ps", name="out_ps")
                    for it in range(IT):
                        nc.tensor.matmul(
                            acc,
                            lhsT=oT_sb[:, it, st * 128:(st + 1) * 128],
                            rhs=wo_sb[:, it, cc * 512:(cc + 1) * 512],
                            start=(it == 0), stop=(it == IT - 1))
                    ob = outp.tile([128, 512], f32, tag="ob", name="ob")
                    nc.vector.tensor_copy(out=ob, in_=acc)
                    nc.sync.dma_start(
                        out=out_d[st * 128:(st + 1) * 128, cc * 512:(cc + 1) * 512],
                        in_=ob)

    nc.compile()
    return nc


@functools.lru_cache(maxsize=1)
def _built():
    return _build()


def _in_maps(hidden_states, Wq, Wk, Wv, Wo):
    bf = ml_dtypes.bfloat16
    maps = []
    for c in range(NCORES):
        b, half = divmod(c, 2)
        sl = slice(half * I, (half + 1) * I)
        maps.append({
            "xt": np.ascontiguousarray(hidden_states[b].T).astype(bf),
            "wq": np.ascontiguousarray(Wq[:, sl]).astype(bf),
            "wk": np.ascontiguousarray(Wk[:, sl]).astype(bf),
            "wv": np.ascontiguousarray(Wv[:, sl]).astype(bf),
            "wo": np.ascontiguousarray(Wo[sl, :]).astype(bf),
        })
    return maps


def kernel(hidden_states, Wq, Wk, Wv, Wo, bo, _trace=False, _trace_kwargs=None):
    from concourse import bass_utils

    nc = _built()
    maps = _in_maps(np.asarray(hidden_states), np.asarray(Wq), np.asarray(Wk),
                    np.asarray(Wv), np.asarray(Wo))
    res = bass_utils.run_bass_kernel_spmd(
        nc, maps, core_ids=list(range(NCORES)), trace=_trace,
        **(_trace_kwargs or {}))
    B = hidden_states.shape[0]
    out = np.empty((B, S, C), np.float32)
    for b in range(B):
        out[b] = res.results[2 * b]["out"] + res.results[2 * b + 1]["out"]
    out += np.asarray(bo, np.float32)
    if _trace:
        return out, res
    return out


# revision 5
# speedup vs baseline: 2.3625x; 2.3625x over previous
"""Multi-head attention (B=4, S=2048, H=16, D=64, C=1024) on 8 NeuronCores.

Sharding: core c handles batch b=c//2 and head-half half=c%2 (8 heads = 512
inner dims).  Each core computes q/k/v projections for its half of the heads,
full softmax attention over S=2048, and a partial output projection through
its 512 rows of Wo.  Host sums the two partials per batch and adds the bias.

Per-core kernel layout (all matmul operands bf16, PSUM accumulation fp32):
  xt    [C=1024, S=2048]   hidden_states[b].T          (host pre-transposed)
  wq/wk/wv [C, I=512]      per-half weight columns
  wo    [I=512, C=1024]    per-half weight rows
  qT,kT [I, S] stored as 4 SBUF tiles [128, 2048]  (head pair per tile)
  v_pad [S, 8*65]          v with a ones column per head (row-sum via matmul)
  scores^T per (pair, qi-chunk, kj-tile): [kj=128, qi=512] via row-tiled
  (K=64) matmul pairs; exp on ScalarE; p@[v|1] accumulated in PSUM over kj.
"""

import functools

import numpy as np
import ml_dtypes

S = 2048          # sequence length
C = 1024          # query dim
I = 512           # inner dims per core (8 heads x 64)
HC = 8            # heads per core
D = 64            # head dim
NCORES = 8
SCALE = D ** -0.5
CT = C // 128     # 8 c-tiles
IT = I // 128     # 4 i-tiles (head pairs)
ST = S // 128     # 16 s-tiles
NQ = S // 512     # 4 qi chunks
VW = D + 1        # 65: v plus ones column


def _build(repeat=1):
    import contextlib

    import concourse.bacc as bacc
    import concourse.tile as tile
    from concourse import mybir

    f32 = mybir.dt.float32
    bf16 = mybir.dt.bfloat16
    Exp = mybir.ActivationFunctionType.Exp

    nc = bacc.Bacc("TRN2", target_bir_lowering=False, debug=False,
                   num_devices=NCORES)

    xt_d = nc.dram_tensor("xt", [C, S], bf16, kind="ExternalInput").ap()
    wq_d = nc.dram_tensor("wq", [C, I], bf16, kind="ExternalInput").ap()
    wk_d = nc.dram_tensor("wk", [C, I], bf16, kind="ExternalInput").ap()
    wv_d = nc.dram_tensor("wv", [C, I], bf16, kind="ExternalInput").ap()
    wo_d = nc.dram_tensor("wo", [I, C], bf16, kind="ExternalInput").ap()
    out_d = nc.dram_tensor("out", [S, C], f32, kind="ExternalOutput").ap()

    with tile.TileContext(nc) as tc:
        with contextlib.ExitStack() as ctx:
            if repeat > 1:
                ctx.enter_context(tc.For_i(0, repeat, 1))
            const = ctx.enter_context(tc.tile_pool(name="const", bufs=1))
            work = ctx.enter_context(tc.tile_pool(name="work", bufs=3))
            outp = ctx.enter_context(tc.tile_pool(name="outp", bufs=3))
            ps_pool = ctx.enter_context(tc.tile_pool(name="ps", bufs=4, space="PSUM"))
            pv_pool = ctx.enter_context(tc.tile_pool(name="pv", bufs=4, space="PSUM"))
            # ---- load inputs -------------------------------------------------
            xt_sb = const.tile([128, CT, S], bf16)
            xt_r = xt_d.rearrange("(t p) s -> p t s", p=128)
            for ct in range(CT):
                nc.sync.dma_start(out=xt_sb[:, ct, :], in_=xt_r[:, ct, :])

            wq_sb = const.tile([128, CT, I], bf16)
            nc.sync.dma_start(out=wq_sb, in_=wq_d.rearrange("(t p) i -> p t i", p=128))
            wk_sb = const.tile([128, CT, I], bf16)
            nc.sync.dma_start(out=wk_sb, in_=wk_d.rearrange("(t p) i -> p t i", p=128))
            wv_sb = const.tile([128, CT, I], bf16)
            nc.sync.dma_start(out=wv_sb, in_=wv_d.rearrange("(t p) i -> p t i", p=128))
            wo_sb = const.tile([128, IT, C], bf16)
            nc.sync.dma_start(out=wo_sb, in_=wo_d.rearrange("(t p) c -> p t c", p=128))

            ones_sb = const.tile([1, 128], bf16)
            nc.vector.memset(ones_sb, 1.0)

            # ---- projections -------------------------------------------------
            qT_sb = const.tile([128, IT, S], bf16)
            kT_sb = const.tile([128, IT, S], bf16)
            v_sb = const.tile([128, ST, HC * VW], bf16)
            # ones column per head for the softmax denominator
            v_ones = v_sb.rearrange("p t (h e) -> p t h e", e=VW)[:, :, :, D:D + 1]
            nc.vector.memset(v_ones, 1.0)

            for (w_sb, o_sb) in ((wq_sb, qT_sb), (wk_sb, kT_sb)):
                for it in range(IT):
                    for nq in range(NQ):
                        acc = ps_pool.tile([128, 512], f32, tag="ps", name="proj_ps")
                        for ct in range(CT):
                            nc.tensor.matmul(
                                acc,
                                lhsT=w_sb[:, ct, it * 128:(it + 1) * 128],
                                rhs=xt_sb[:, ct, nq * 512:(nq + 1) * 512],
                                start=(ct == 0), stop=(ct == CT - 1))
                        nc.vector.tensor_copy(
                            out=o_sb[:, it, nq * 512:(nq + 1) * 512], in_=acc)

            v_main = v_sb.rearrange("p t (h e) -> p t h e", e=VW)[:, :, :, 0:D]
            for st in range(ST):
                acc = ps_pool.tile([128, 512], f32, tag="ps", name="v_ps")
                for ct in range(CT):
                    nc.tensor.matmul(
                        acc,
                        lhsT=xt_sb[:, ct, st * 128:(st + 1) * 128],
                        rhs=wv_sb[:, ct, :],
                        start=(ct == 0), stop=(ct == CT - 1))
                nc.vector.tensor_copy(
                    out=v_main[:, st],
                    in_=acc.rearrange("p (h d) -> p h d", d=D))

            # ---- attention ---------------------------------------------------
            oT_sb = const.tile([128, IT, S], bf16)
            for hp in range(IT):
                hA, hB = 2 * hp, 2 * hp + 1
                for nq in range(NQ):
                    qs = slice(nq * 512, (nq + 1) * 512)
                    oA = pv_pool.tile([VW, 512], f32, tag="pv", name="oA")
                    oB = pv_pool.tile([VW, 512], f32, tag="pv", name="oB")
                    for kt in range(ST):
                        ks = slice(kt * 128, (kt + 1) * 128)
                        sA = ps_pool.tile([128, 512], f32, tag="ps", name="sA")
                        sB = ps_pool.tile([128, 512], f32, tag="ps", name="sB")
                        nc.tensor.matmul(
                            sA, lhsT=kT_sb[0:64, hp, ks], rhs=qT_sb[0:64, hp, qs],
                            start=True, stop=True, tile_position=(0, 0))
                        nc.tensor.matmul(
                            sB, lhsT=kT_sb[64:128, hp, ks], rhs=qT_sb[64:128, hp, qs],
                            start=True, stop=True, tile_position=(64, 0))
                        pA = work.tile([128, 512], bf16, tag="p", name="pA")
                        pB = work.tile([128, 512], bf16, tag="p", name="pB")
                        nc.scalar.activation(out=pA, in_=sA, func=Exp, scale=SCALE)
                        nc.scalar.activation(out=pB, in_=sB, func=Exp, scale=SCALE)
                        nc.tensor.matmul(
                            oA, lhsT=v_sb[:, kt, hA * VW:(hA + 1) * VW], rhs=pA,
                            start=(kt == 0), stop=(kt == ST - 1))
                        nc.tensor.matmul(
                            oB, lhsT=v_sb[:, kt, hB * VW:(hB + 1) * VW], rhs=pB,
                            start=(kt == 0), stop=(kt == ST - 1))
                    # normalize: recip of the denominator row, broadcast across
                    # the 64 head dims via a K=1 matmul, multiply on VectorE
                    rA = work.tile([1, 512], mybir.dt.float32, tag="recip", name="rA")
                    rB = work.tile([1, 512], mybir.dt.float32, tag="recip", name="rB")
                    nc.vector.reciprocal(out=rA, in_=oA[D:VW, :])
                    nc.vector.reciprocal(out=rB, in_=oB[D:VW, :])
                    rAb = work.tile([1, 512], bf16, tag="recipb", name="rAb")
                    rBb = work.tile([1, 512], bf16, tag="recipb", name="rBb")
                    nc.vector.tensor_copy(out=rAb, in_=rA)
                    nc.vector.tensor_copy(out=rBb, in_=rB)
                    bc = ps_pool.tile([128, 512], f32, tag="ps", name="bc")
                    nc.tensor.matmul(bc[0:64, :], lhsT=ones_sb[0:1, 0:64],
                                     rhs=rAb, start=True, stop=True,
                                     tile_position=(0, 0))
                    nc.tensor.matmul(bc[64:128, :], lhsT=ones_sb[0:1, 0:64],
                                     rhs=rBb, start=True, stop=True,
                                     tile_position=(0, 64))
                    tA = work.tile([64, 512], mybir.dt.float32, tag="t", name="tA")
                    tB = work.tile([64, 512], mybir.dt.float32, tag="t", name="tB")
                    nc.vector.tensor_copy(out=tA, in_=oA[0:D, :])
                    nc.vector.tensor_copy(out=tB, in_=oB[0:D, :])
                    nc.vector.tensor_mul(
                        out=oT_sb[0:64, hp, qs], in0=tA, in1=bc[0:64, :])
                    nc.vector.tensor_mul(
                        out=oT_sb[64:128, hp, qs], in0=tB, in1=bc[64:128, :])

            # ---- output projection ------------------------------------------
            for st in range(ST):
                for cc in range(2):
                    acc = ps_pool.tile([128, 512], f32, tag="ps", name="out_ps")
                    for it in range(IT):
                        nc.tensor.matmul(
                            acc,
                            lhsT=oT_sb[:, it, st * 128:(st + 1) * 128],
                            rhs=wo_sb[:, it, cc * 512:(cc + 1) * 512],
                            start=(it == 0), stop=(it == IT - 1))
                    ob = outp.tile([128, 512], f32, tag="ob", name="ob")
                    nc.vector.tensor_copy(out=ob, in_=acc)
                    nc.sync.dma_start(
                        out=out_d[st * 128:(st + 1) * 128, cc * 512:(cc + 1) * 512],
                        in_=ob)

    nc.compile()
    return nc


@functools.lru_cache(maxsize=4)
def _built(repeat=1):
    return _build(repeat)


def _in_maps(hidden_states, Wq, Wk, Wv, Wo):
    bf = ml_dtypes.bfloat16
    maps = []
    for c in range(NCORES):
        b, half = divmod(c, 2)
        sl = slice(half * I, (half + 1) * I)
        maps.append({
            "xt": np.ascontiguousarray(hidden_states[b].T).astype(bf),
            "wq": np.ascontiguousarray(Wq[:, sl]).astype(bf),
            "wk": np.ascontiguousarray(Wk[:, sl]).astype(bf),
            "wv": np.ascontiguousarray(Wv[:, sl]).astype(bf),
            "wo": np.ascontiguousarray(Wo[sl, :]).astype(bf),
        })
    return maps


def kernel(hidden_states, Wq, Wk, Wv, Wo, bo, _trace=False, _trace_kwargs=None):
    from concourse import bass_utils

    nc = _built()
    maps = _in_maps(np.asarray(hidden_states), np.asarray(Wq), np.asarray(Wk),
                    np.asarray(Wv), np.asarray(Wo))
    res = bass_utils.run_bass_kernel_spmd(
        nc, maps, core_ids=list(range(NCORES)), trace=_trace,
        **(_trace_kwargs or {}))
    B = hidden_states.shape[0]
    out = np.empty((B, S, C), np.float32)
    for b in range(B):
        out[b] = res.results[2 * b]["out"] + res.results[2 * b + 1]["out"]
    out += np.asarray(bo, np.float32)
    if _trace:
        return out, res
    return out


# revision 40
# speedup vs baseline: 4.8765x; 2.0641x over previous
"""Multi-head attention (B=4, S=2048, H=16, D=64, C=1024) on 8 NeuronCores.

Sharding: core c handles batch b=c//2 and head-half half=c%2 (8 heads = 512
inner dims).  Each core computes q/k/v projections for its half of the heads,
full softmax attention over S=2048, and a partial output projection through
its 512 rows of Wo.  Host sums the two partials per batch and adds the bias.

Per-core kernel layout (all matmul operands bf16, PSUM accumulation fp32):
  xt    [C=1024, S=2048]   hidden_states[b].T          (host pre-transposed)
  wq/wk/wv [C, I=512]      per-half weight columns
  wo    [I=512, C=1024]    per-half weight rows
  qT,kT [I, S] stored as 4 SBUF tiles [128, 2048]  (head pair per tile)
  v_pad [S, 8*65]          v with a ones column per head (row-sum via matmul)
  scores^T per (pair, qi-chunk, kj-tile): [kj=128, qi=512] via row-tiled
  (K=64) matmul pairs; exp on ScalarE; p@[v|1] accumulated in PSUM over kj.
"""

import functools

import numpy as np
import ml_dtypes

S = 2048          # sequence length
C = 1024          # query dim
I = 512           # inner dims per core (8 heads x 64)
HC = 8            # heads per core
D = 64            # head dim
NCORES = 8
SCALE = D ** -0.5
CT = C // 128     # 8 c-tiles
IT = I // 128     # 4 i-tiles (head pairs)
ST = S // 128     # 16 s-tiles
NQ = S // 512     # 4 qi chunks
VW = D + 1        # 65: v plus ones column


def _build(repeat=1, phases="dma,proj,attn,outproj", fused_exp=True):
    import contextlib

    import concourse.bacc as bacc
    import concourse.tile as tile
    from concourse import mybir

    f32 = mybir.dt.float32
    bf16 = mybir.dt.bfloat16
    Exp = mybir.ActivationFunctionType.Exp

    nc = bacc.Bacc("TRN2", target_bir_lowering=False, debug=False,
                   num_devices=NCORES)

    # All inputs are host-pre-shuffled to partition-major [128, ...] layouts so
    # every DMA reads long contiguous runs per partition.
    xt_d = nc.dram_tensor("xt", [128, CT * S], bf16, kind="ExternalInput").ap()
    wq_d = nc.dram_tensor("wq", [128, CT * I], bf16, kind="ExternalInput").ap()
    wk_d = nc.dram_tensor("wk", [128, CT * I], bf16, kind="ExternalInput").ap()
    wv_d = nc.dram_tensor("wv", [128, CT * I], bf16, kind="ExternalInput").ap()
    wo_d = nc.dram_tensor("wo", [128, IT * C], bf16, kind="ExternalInput").ap()
    out_d = nc.dram_tensor("out", [S, C], f32, kind="ExternalOutput").ap()

    with tile.TileContext(nc) as tc:
        with contextlib.ExitStack() as ctx:
            if repeat > 1:
                ctx.enter_context(tc.For_i(0, repeat, 1))
            const = ctx.enter_context(tc.tile_pool(name="const", bufs=1))
            work = ctx.enter_context(tc.tile_pool(name="work", bufs=3))
            outp = ctx.enter_context(tc.tile_pool(name="outp", bufs=3))
            ps_pool = ctx.enter_context(tc.tile_pool(name="ps", bufs=3, space="PSUM"))
            pv_pool = ctx.enter_context(tc.tile_pool(name="pv", bufs=2, space="PSUM"))
            # ---- load inputs (contiguous per partition) ---------------------
            xt_sb = const.tile([128, CT, S], bf16)
            xt_r = xt_d.rearrange("p (t s) -> p t s", s=S)
            for q4 in range(4):
                nc.sync.dma_start(out=xt_sb[:, q4 * 2:(q4 + 1) * 2, :],
                                  in_=xt_r[:, q4 * 2:(q4 + 1) * 2, :])

            wq_sb = const.tile([128, CT, I], bf16)
            nc.scalar.dma_start(out=wq_sb, in_=wq_d.rearrange("p (t i) -> p t i", i=I))
            wk_sb = const.tile([128, CT, I], bf16)
            nc.scalar.dma_start(out=wk_sb, in_=wk_d.rearrange("p (t i) -> p t i", i=I))
            wv_sb = const.tile([128, CT, I], bf16)
            nc.scalar.dma_start(out=wv_sb, in_=wv_d.rearrange("p (t i) -> p t i", i=I))
            wo_sb = const.tile([128, IT, C], bf16)
            nc.scalar.dma_start(out=wo_sb, in_=wo_d.rearrange("p (t c) -> p t c", c=C))

            ones_sb = const.tile([1, 128], bf16)
            nc.vector.memset(ones_sb, 1.0)

            phs = set(phases.split(","))

            # ---- projections -------------------------------------------------
            qT_sb = const.tile([128, IT, S], bf16)
            kT_sb = const.tile([128, IT, S], bf16)
            v_sb = const.tile([128, ST, HC * VW], bf16)
            # ones columns per head (softmax denominator): contiguous memset of
            # the whole tile — the v copies then overwrite the 64 data columns.
            # (A strided single-element memset here measures ~78us on HW.)
            nc.vector.memset(v_sb, 1.0)

            oT_sb = const.tile([128, IT, S], bf16)

            # ---- projections -------------------------------------------------
            if "proj" in phs:
                def proj_qk_acc(it, which, nq):
                    w_sb, o_sb = ((wq_sb, qT_sb), (wk_sb, kT_sb))[which]
                    acc = ps_pool.tile([128, 1024], f32, tag="ps",
                                       name="proj_ps")
                    for ct in range(CT):
                        for h2 in range(2):
                            nc.tensor.matmul(
                                acc[:, h2 * 512:(h2 + 1) * 512],
                                lhsT=w_sb[:, ct, it * 128:(it + 1) * 128],
                                rhs=xt_sb[:, ct, nq * 1024 + h2 * 512:
                                          nq * 1024 + (h2 + 1) * 512],
                                start=(ct == 0), stop=(ct == CT - 1))
                    nc.vector.tensor_copy(
                        out=o_sb[:, it, nq * 1024:(nq + 1) * 1024], in_=acc)

                def proj_qk(it):
                    for which in range(2):
                        for nq in range(S // 1024):
                            proj_qk_acc(it, which, nq)

                v_main = v_sb.rearrange("p t (h e) -> p t h e", e=VW)[:, :, :, 0:D]

                def proj_v(st):
                    acc = ps_pool.tile([128, 512], f32, tag="ps", name="v_ps")
                    for ct in range(CT):
                        nc.tensor.matmul(
                            acc,
                            lhsT=xt_sb[:, ct, st * 128:(st + 1) * 128],
                            rhs=wv_sb[:, ct, :],
                            start=(ct == 0), stop=(ct == CT - 1))
                    nc.vector.tensor_copy(
                        out=v_main[:, st],
                        in_=acc.rearrange("p (h d) -> p h d", d=D))

            # ---- attention + interleaved output projection ------------------
            def attn_chunk(hp, nq, extra=None):
                hA, hB = 2 * hp, 2 * hp + 1
                qs = slice(nq * 512, (nq + 1) * 512)
                oA = pv_pool.tile([VW, 512], f32, tag="pv", name="oA")
                oB = pv_pool.tile([VW, 512], f32, tag="pv", name="oB")
                for kt in range(ST):
                    if extra is not None:
                        extra(kt)
                    ks = slice(kt * 128, (kt + 1) * 128)
                    sAB = ps_pool.tile([128, 1024], f32, tag="ps", name="sAB")
                    sA, sB = sAB[:, 0:512], sAB[:, 512:1024]
                    nc.tensor.matmul(
                        sA, lhsT=kT_sb[0:64, hp, ks], rhs=qT_sb[0:64, hp, qs],
                        start=True, stop=True, tile_position=(0, 0))
                    nc.tensor.matmul(
                        sB, lhsT=kT_sb[64:128, hp, ks], rhs=qT_sb[64:128, hp, qs],
                        start=True, stop=True, tile_position=(64, 0))
                    pAB = work.tile([128, 1024], bf16, tag="p", bufs=8, name="pAB")
                    nc.scalar.activation(out=pAB, in_=sAB, func=Exp, scale=SCALE)
                    nc.tensor.matmul(
                        oA, lhsT=v_sb[:, kt, hA * VW:(hA + 1) * VW],
                        rhs=pAB[:, 0:512],
                        start=(kt == 0), stop=(kt == ST - 1))
                    nc.tensor.matmul(
                        oB, lhsT=v_sb[:, kt, hB * VW:(hB + 1) * VW],
                        rhs=pAB[:, 512:1024],
                        start=(kt == 0), stop=(kt == ST - 1))
                # normalize: recip of the denominator row, broadcast across the
                # 64 head dims via a K=1 matmul, multiply on VectorE
                # normalize: reciprocal of the denominator row (DVE), broadcast
                # across partitions on the idle GpSimd engine, multiply on DVE
                rA = work.tile([1, 512], mybir.dt.float32, tag="recip", name="rA")
                rB = work.tile([1, 512], mybir.dt.float32, tag="recip", name="rB")
                nc.vector.reciprocal(out=rA, in_=oA[D:VW, :])
                nc.vector.reciprocal(out=rB, in_=oB[D:VW, :])
                bcA = work.tile([64, 512], mybir.dt.float32, tag="bcs", name="bcA")
                bcB = work.tile([64, 512], mybir.dt.float32, tag="bcs", name="bcB")
                nc.gpsimd.partition_broadcast(bcA, rA)
                nc.gpsimd.partition_broadcast(bcB, rB)
                nc.vector.tensor_mul(
                    out=oT_sb[0:64, hp, qs], in0=oA[0:D, :], in1=bcA)
                nc.vector.tensor_mul(
                    out=oT_sb[64:128, hp, qs], in0=oB[0:D, :], in1=bcB)

            def outproj_tile(st):
                acc = ps_pool.tile([128, 1024], f32, tag="ps", name="out_ps")
                for it in range(IT):
                    for h2 in range(2):
                        nc.tensor.matmul(
                            acc[:, h2 * 512:(h2 + 1) * 512],
                            lhsT=oT_sb[:, it, st * 128:(st + 1) * 128],
                            rhs=wo_sb[:, it, h2 * 512:(h2 + 1) * 512],
                            start=(it == 0), stop=(it == IT - 1))
                ob = outp.tile([128, 1024], f32, tag="ob", name="ob")
                nc.vector.tensor_copy(out=ob, in_=acc)
                nc.sync.dma_start(out=out_d[st * 128:(st + 1) * 128, :], in_=ob)

            if "attn" in phs:
                # pair-major chunk order; qk projections for the next pair and
                # v tiles are woven into chunk kt-steps so PE slack under the
                # ACT-bound exp stream absorbs them.
                if "proj" in phs:
                    proj_qk(0)
                    for st in range(4):
                        proj_v(st)

                def make_extra(hp, nq):
                    if "proj" not in phs:
                        return None
                    def extra(kt):
                        if hp == 0 and nq == 0 and 4 + kt < ST:
                            proj_v(4 + kt)
                        if hp < IT - 1 and nq in (1, 2) and kt in (3, 11):
                            acc_idx = (nq - 1) * 2 + (0 if kt == 3 else 1)
                            proj_qk_acc(hp + 1, acc_idx // 2, acc_idx % 2)
                    return extra

                for hp in range(IT):
                    for nq in range(NQ):
                        attn_chunk(hp, nq, make_extra(hp, nq))
                        if hp == IT - 1 and "outproj" in phs:
                            for st in range(4 * nq, 4 * (nq + 1)):
                                outproj_tile(st)
            else:
                if "proj" in phs:
                    for it in range(IT):
                        proj_qk(it)
                    for st in range(ST):
                        proj_v(st)
                if "outproj" in phs:
                    for st in range(ST):
                        outproj_tile(st)

    nc.compile()
    return nc


@functools.lru_cache(maxsize=8)
def _built(repeat=1, phases="dma,proj,attn,outproj", fused_exp=True):
    return _build(repeat, phases, fused_exp)


def _pm(a):
    """[T*128, F] -> partition-major [128, T*F] (bf16)."""
    T = a.shape[0] // 128
    return np.ascontiguousarray(
        a.reshape(T, 128, a.shape[1]).swapaxes(0, 1).reshape(128, -1)
    ).astype(ml_dtypes.bfloat16)


def _in_maps(hidden_states, Wq, Wk, Wv, Wo):
    maps = []
    for c in range(NCORES):
        b, half = divmod(c, 2)
        sl = slice(half * I, (half + 1) * I)
        maps.append({
            "xt": _pm(np.ascontiguousarray(hidden_states[b].T)),
            "wq": _pm(Wq[:, sl]),
            "wk": _pm(Wk[:, sl]),
            "wv": _pm(Wv[:, sl]),
            "wo": _pm(Wo[sl, :]),
        })
    return maps


@functools.lru_cache(maxsize=1)
def _runner():
    """Compile the SPMD program once and return a function
    maps -> list of per-core output dicts."""
    import jax
    from jax.sharding import Mesh, PartitionSpec, NamedSharding
    from jax.experimental.shard_map import shard_map

    import concourse.mybir as mybir
    from concourse.bass2jax import (
        _bass_exec_p, install_neuronx_cc_hook, partition_id_tensor)

    nc = _built()
    install_neuronx_cc_hook()
    partition_name = nc.partition_id_tensor.name if nc.partition_id_tensor else None

    in_names, out_names, out_avals, zero_outs = [], [], [], []
    for alloc in nc.m.functions[0].allocations:
        if not isinstance(alloc, mybir.MemoryLocationSet):
            continue
        name = alloc.memorylocations[0].name
        if alloc.kind == "ExternalInput":
            if name != partition_name:
                in_names.append(name)
        elif alloc.kind == "ExternalOutput":
            out_names.append(name)
            shape = tuple(alloc.tensor_shape)
            dtype = mybir.dt.np(alloc.dtype)
            out_avals.append(jax.core.ShapedArray(shape, dtype))
            zero_outs.append(np.zeros(shape, dtype))
    n_params = len(in_names)
    all_in_names = in_names + out_names
    if partition_name is not None:
        all_in_names = all_in_names + [partition_name]

    def _body(*args):
        operands = list(args)
        if partition_name is not None:
            operands.append(partition_id_tensor())
        return tuple(_bass_exec_p.bind(
            *operands,
            out_avals=tuple(out_avals),
            in_names=tuple(all_in_names),
            out_names=tuple(out_names),
            lowering_input_output_aliases=(),
            sim_require_finite=True,
            sim_require_nnan=True,
            nc=nc,
        ))

    devices = jax.devices()[:NCORES]
    mesh = Mesh(np.asarray(devices), ("core",))
    in_specs = (PartitionSpec("core"),) * (n_params + len(out_names))
    out_specs = (PartitionSpec("core"),) * len(out_names)
    sharded = jax.jit(
        shard_map(_body, mesh=mesh, in_specs=in_specs, out_specs=out_specs,
                  check_rep=False),
        keep_unused=True,
    )
    sharding = NamedSharding(mesh, PartitionSpec("core"))
    dev_zero = [jax.device_put(
        np.zeros((NCORES * z.shape[0], *z.shape[1:]), z.dtype), sharding)
        for z in zero_outs]

    def run(maps):
        concat_in = [np.concatenate([np.asarray(maps[c][n]) for c in range(NCORES)],
                                    axis=0) for n in in_names]
        dev_in = [jax.device_put(a, sharding) for a in concat_in]
        out_arrs = sharded(*dev_in, *dev_zero)
        return [
            {n: np.asarray(out_arrs[i]).reshape(NCORES, *out_avals[i].shape)[c]
             for i, n in enumerate(out_names)}
            for c in range(NCORES)
        ]

    return run


def kernel(hidden_states, Wq, Wk, Wv, Wo, bo):
    maps = _in_maps(np.asarray(hidden_states), np.asarray(Wq), np.asarray(Wk),
                    np.asarray(Wv), np.asarray(Wo))
    results = _runner()(maps)
    B = hidden_states.shape[0]
    out = np.empty((B, S, C), np.float32)
    for b in range(B):
        out[b] = results[2 * b]["out"] + results[2 * b + 1]["out"]
    out += np.asarray(bo, np.float32)
    return out
